# revision 17
# baseline (speedup 1.0000x reference)
# Trainium2 Bass kernel for nn_DiversityLoss (segment_reduce).
#
# reference:
#   sums   = segment_sum(embeddings, labels, C)        # [C, D]
#   counts = segment_sum(ones, labels, C)              # [C]
#   return -mean(var(sums / counts, axis=0, ddof=1))
#
# Strategy (sorted class-pure tiles, fp8, PE one-hot routing; v2 adds a DVE
# pair-add offload):
#   - Host sorts rows by label, pads each class to a multiple of 128 rows,
#     quantizes embeddings to fp8e4 (9.7e-4 final rel err vs the 2e-2 gate;
#     exact fp32 PSUM accumulation), and deals classes 125-per-core ranked by
#     tile count so every core has an identical tile schedule (SPMD).
#   - Each 128-row tile is class-pure. The v1 kernel issued one matmul per
#     tile; profile showed the PE gets duty-cycle throttled (ham k=4/n=8
#     windows from ~36us) to ~52ns/tile, just below the 50ns/tile DMA rate,
#     building a ~150-tile backlog cleared 5us after the stream ends.
#   - v2: the (otherwise idle) DVE pre-adds pairs of same-class tiles in fp8
#     (first half of each class run + second half, both contiguous), halving
#     the PE stream; PSUM flush copies moved to the idle gpsimd engine.
#   - Stationary shrunk from 128KB to an 8KB [128, 63] strip (col 31 ones):
#     slice [31-j, 31+32-j) puts the ones-column at local position j. It
#     rides the sync ring FIRST so the first matmul can start ~1.2us earlier.
#   - Each class layer (32 classes) accumulates in its own PSUM bank so
#     flushing a finished layer never blocks the PE writing later layers.
#   - All input chunks ride the sync HWDGE ring back-to-back, dispatched up
#     front; every chunk stays resident in SBUF so DMA never waits on
#     compute. Last chunks are small and unpaired so the tail after the
#     final byte is ~0.3us of direct matmuls.
#   - Host sums the 4 column-group replicas per class, divides by exact
#     bincount counts, computes the variance in float64.

import numpy as np
import ml_dtypes

D = 128
C = 1000
CORES = 8
CPC = C // CORES  # 125 classes per core

TRACE = False
TRACE_KWARGS = {}
LAST_RESULT = None

_cache = {}

# chunks with index >= len(sizes)-UNPAIRED_TAIL are not DVE-paired
UNPAIRED_TAIL = 3


PAIR_FRAC = 0.40  # fraction of tiles consumed by DVE pair-adds


def _plan(T_pos):
    """Build the per-chunk schedule: DVE pair-adds and the MM sequence.

    DVE fp8 tensor_add measured ~105 Gelem/s (~156ns per paired output
    tile), so pairing everything makes DVE the bottleneck (95us); the
    PE under its duty-cycle throttle runs ~52-58ns/MM, so pairing
    nothing leaves the PE just below DMA rate (v1's 5us tail). Pair an
    exact PAIR_FRAC of tiles, and schedule each chunk's scratch MMs one
    chunk LATE so the PE never waits on the DVE (the adds get a full
    ~6us chunk period of slack).
    """
    tiles = [p for p in range(CPC) for _ in range(T_pos[p])]
    NT = len(tiles)
    sizes = _chunk_sizes(NT)
    n_ch = len(sizes)

    chunks = []
    a = 0
    seen = 0
    paired = 0
    pend_mms = []  # scratch MMs deferred to the next chunk
    for ci, sz in enumerate(sizes):
        ct = tiles[a : a + sz]
        runs = []
        s = 0
        for i in range(1, sz + 1):
            if i == sz or ct[i] != ct[s]:
                runs.append((s, i - s, ct[s]))  # local offset, length, class
                s = i
        dve = []  # (out_off, k, in0_off, in1_off)
        mms = list(pend_mms)  # previous chunk's scratch MMs: data ready
        pend_mms = []
        off = 0
        pair = ci < n_ch - UNPAIRED_TAIL
        for s, m, p in runs:
            seen += m
            k = 0
            if pair:
                want = int(PAIR_FRAC * seen) - paired
                k = max(0, min(m // 2, want // 2))
            paired += 2 * k
            if k:
                dve.append((off, k, s, s + k))
            for i in range(s + 2 * k, s + m):
                mms.append(("d", ci, i, p))
            for i in range(k):
                pend_mms.append(("s", ci, off + i, p))
            off += k
        chunks.append((sz, dve, mms, off))
        a += sz
    if pend_mms:
        chunks[-1] = (
            chunks[-1][0],
            chunks[-1][1],
            chunks[-1][2] + pend_mms,
            chunks[-1][3],
        )
    return sizes, chunks


def _build_module(T_pos):
    import concourse.mybir as mybir
    import concourse.tile as tile
    from concourse import bacc

    f8 = mybir.dt.float8e4
    f32 = mybir.dt.float32

    sizes, chunks = _plan(T_pos)
    max_scr = max(c[3] for c in chunks)

    nc = bacc.Bacc(
        "TRN2",
        target_bir_lowering=False,
        debug=False,
        enable_asserts=False,
        num_devices=CORES,
    )
    emb_ds = [
        nc.dram_tensor(f"emb{i}", [128, sz * D], f8, kind="ExternalInput")
        for i, sz in enumerate(sizes)
    ]
    w_d = nc.dram_tensor("w", [128, 63], f8, kind="ExternalInput")
    out_d = nc.dram_tensor("out", [128, 512], f32, kind="ExternalOutput")

    # global MM order -> (key=(r,l), first/last) for PSUM start/stop flags
    seq = []
    for sz, dve, mms, n_scr in chunks:
        seq.extend(mms)
    first = {}
    last = {}
    layer_last = {}
    for t, (kind, src, off, p) in enumerate(seq):
        key = (t % 4, p // 32)
        first.setdefault(key, t)
        last[key] = t
        layer_last[p // 32] = t
    flush_after = {layer_last[l]: l for l in layer_last}

    from contextlib import ExitStack

    with tile.TileContext(nc) as tc, ExitStack() as stack:
        consts = stack.enter_context(tc.tile_pool(name="consts", bufs=1))
        # one exact-size pool per chunk: a single pool would allocate
        # len(sizes) slots of the max chunk size and overflow SBUF
        epools = [
            stack.enter_context(tc.tile_pool(name=f"ebuf{i}", bufs=1))
            for i in range(len(sizes))
        ]
        scr = stack.enter_context(tc.tile_pool(name="scr", bufs=3))
        psum = stack.enter_context(
            tc.tile_pool(name="psum", bufs=1, space="PSUM")
        )
        outb = stack.enter_context(tc.tile_pool(name="outb", bufs=1))
        if True:
            w_t = consts.tile([128, 63], f8)
            # stationary strip rides the sync ring FIRST (8KB, ~25ns)
            nc.sync.dma_start(out=w_t[:], in_=w_d[:])

            ps_l = [
                psum.tile([128, 512], f32, name=f"ps{i}") for i in range(4)
            ]
            out_t = outb.tile([128, 512], f32)

            ets = []
            # input chunks alternate between the sync and scalar HWDGE
            # rings; the two rings split the ~330-358 GB/s per-core HBM
            # share, and the parallel dispatch shortens the ramp
            for ch, sz in enumerate(sizes):
                et = epools[ch].tile([128, sz * D], f8, tag=f"et{ch}")
                ets.append(et)
                eng = nc.sync if ch % 2 == 0 else nc.scalar
                eng.dma_start(out=et[:], in_=emb_ds[ch][:])

            t = 0
            sts = {}
            for ci, (sz, dve, mms, n_scr) in enumerate(chunks):
                et = ets[ci]
                if dve:
                    st = scr.tile([128, max(max_scr, 1) * D], f8, tag="scr")
                    sts[ci] = st
                for out_off, k, i0, i1 in dve:
                    nc.vector.tensor_add(
                        out=st[:, out_off * D : (out_off + k) * D],
                        in0=et[:, i0 * D : (i0 + k) * D],
                        in1=et[:, i1 * D : (i1 + k) * D],
                    )
                for kind, src_ci, off, p in mms:
                    r = t % 4
                    l = p // 32
                    j32 = p % 32
                    key = (r, l)
                    src = ets[src_ci] if kind == "d" else sts[src_ci]
                    nc.tensor.matmul(
                        ps_l[l][32 * r : 32 * r + 32, 0:128],
                        lhsT=w_t[:, 31 - j32 : 63 - j32],
                        rhs=src[:, off * D : (off + 1) * D],
                        start=(first[key] == t),
                        stop=(last[key] == t),
                        tile_position=(0, 32 * r),
                    )
                    if t in flush_after:
                        l2 = flush_after[t]
                        nc.vector.tensor_copy(
                            out=out_t[:, 128 * l2 : 128 * (l2 + 1)],
                            in_=ps_l[l2][:, 0:128],
                        )
                        nc.scalar.dma_start(
                            out=out_d[:, 128 * l2 : 128 * (l2 + 1)],
                            in_=out_t[:, 128 * l2 : 128 * (l2 + 1)],
                        )
                    t += 1

    nc.compile()
    return nc


def _schedule(counts):
    T_c = -(-counts // 128)  # ceil
    rank = np.argsort(-T_c, kind="stable")  # class ids, tile count descending
    T_pos = T_c[rank[np.arange(CPC) * 8]]  # max of each octet
    return rank, tuple(int(x) for x in T_pos)


def _chunk_sizes(NT):
    # small head (fast first matmul), ~2MB middle chunks (DMA efficiency),
    # small unpaired tail (minimal PE trailing after the last chunk lands).
    head = [4, 28, 92]
    tail = [64, 16, 8, 4]
    mid_total = NT - sum(head) - sum(tail)
    n_mid = max(1, round(mid_total / 132))
    base = mid_total // n_mid
    mid = [base + (1 if i < mid_total % n_mid else 0) for i in range(n_mid)]
    return head + mid + tail


def kernel(embeddings, labels):
    global LAST_RESULT
    from concourse.bass_utils import run_bass_kernel_spmd

    embeddings = np.asarray(embeddings)
    labels = np.asarray(labels).astype(np.int64)
    N = labels.shape[0]

    counts = np.bincount(labels, minlength=C)
    rank, T_pos = _schedule(counts)
    NT = int(sum(T_pos))
    sizes = _chunk_sizes(NT)

    key = T_pos
    if key not in _cache:
        _cache[key] = _build_module(list(T_pos))
    nc = _cache[key]

    # ---- host layout: sorted, class-padded, per-core ----
    embq = embeddings.astype(ml_dtypes.float8_e4m3)
    embq_ext = np.zeros((N + 1, D), dtype=ml_dtypes.float8_e4m3)
    embq_ext[:N] = embq
    order = np.argsort(labels, kind="stable")
    cls_start = np.zeros(C + 1, dtype=np.int64)
    np.cumsum(counts, out=cls_start[1:])

    slot_base = np.zeros(CPC + 1, dtype=np.int64)
    np.cumsum(np.asarray(T_pos, dtype=np.int64) * 128, out=slot_base[1:])

    w = np.zeros((128, 63), dtype=ml_dtypes.float8_e4m3)
    w[:, 31] = 1.0

    in_maps = []
    for k in range(CORES):
        idx = np.full(NT * 128, N, dtype=np.int64)
        for p in range(CPC):
            c = rank[8 * p + k]
            n = counts[c]
            idx[slot_base[p] : slot_base[p] + n] = order[
                cls_start[c] : cls_start[c] + n
            ]
        ec = embq_ext[idx].reshape(NT, 128, D)  # [tile, row, d] fp8
        m = {"w": w}
        a = 0
        for i, sz in enumerate(sizes):
            m[f"emb{i}"] = np.ascontiguousarray(
                ec[a : a + sz].transpose(1, 0, 2)
            ).reshape(128, sz * D)
            a += sz
        in_maps.append(m)

    res = run_bass_kernel_spmd(
        nc,
        in_maps,
        core_ids=list(range(CORES)),
        trace=TRACE,
        **TRACE_KWARGS,
    )
    LAST_RESULT = res

    # ---- host combine: sum 4 colgroup replicas, then means/variance ----
    sums = np.zeros((C, D), dtype=np.float64)
    for k in range(CORES):
        o = res.results[k]["out"].astype(np.float64)
        # [r=4, j32=32, l=4, d=128] -> sum over r -> [l, j32, d] -> [p, d]
        s_all = o.reshape(4, 32, 4, 128).sum(axis=0).transpose(1, 0, 2)
        s_all = s_all.reshape(CPC + 3, D)[:CPC]
        sums[rank[np.arange(CPC) * 8 + k]] = s_all
    means = sums / counts[:, None]
    mu = means.mean(axis=0)
    var = ((means - mu) ** 2).sum(axis=0) / (C - 1)
    return np.float32(-var.mean())


# revision 18
# speedup vs baseline: 1.1016x; 1.1016x over previous
# Trainium2 Bass kernel for nn_DiversityLoss (segment_reduce).
#
# reference:
#   sums   = segment_sum(embeddings, labels, C)        # [C, D]
#   counts = segment_sum(ones, labels, C)              # [C]
#   return -mean(var(sums / counts, axis=0, ddof=1))
#
# Strategy (sorted class-pure tiles, fp8, PE one-hot routing, with a
# DVE+gpsimd pair-add offload to keep the throttled PE under the DMA rate):
#   - Host sorts rows by label, pads each class to a multiple of 128 rows,
#     quantizes embeddings to fp8e4 (9.7e-4 final rel err vs the 2e-2 gate;
#     exact fp32 PSUM accumulation), and deals classes 125-per-core ranked by
#     tile count so every core has an identical tile schedule (SPMD).
#   - Each 128-row tile is class-pure. The v1 kernel issued one matmul per
#     tile; profile showed the PE gets duty-cycle throttled (ham k=4/n=8
#     windows from ~36us) to ~52ns/tile, just below the 50ns/tile DMA rate,
#     building a ~150-tile backlog cleared 5us after the stream ends.
#   - v2: the (otherwise idle) DVE pre-adds pairs of same-class tiles in fp8
#     (first half of each class run + second half, both contiguous), halving
#     the PE stream; PSUM flush copies moved to the idle gpsimd engine.
#   - Stationary shrunk from 128KB to an 8KB [128, 63] strip (col 31 ones):
#     slice [31-j, 31+32-j) puts the ones-column at local position j. It
#     rides the sync ring FIRST so the first matmul can start ~1.2us earlier.
#   - Each class layer (32 classes) accumulates in its own PSUM bank so
#     flushing a finished layer never blocks the PE writing later layers.
#   - All input chunks ride the sync HWDGE ring back-to-back, dispatched up
#     front; every chunk stays resident in SBUF so DMA never waits on
#     compute. Last chunks are small and unpaired so the tail after the
#     final byte is ~0.3us of direct matmuls.
#   - Host sums the 4 column-group replicas per class, divides by exact
#     bincount counts, computes the variance in float64.

import numpy as np
import ml_dtypes

D = 128
C = 1000
CORES = 8
CPC = C // CORES  # 125 classes per core

TRACE = False
TRACE_KWARGS = {}
LAST_RESULT = None

_cache = {}

# chunks with index >= len(sizes)-UNPAIRED_TAIL are not DVE-paired
UNPAIRED_TAIL = 3


PAIR_FRAC_V = 0.40  # fraction of tiles pair-added on the vector engine
PAIR_FRAC_G = 0.10  # additional fraction pair-added on gpsimd


def _plan(T_pos):
    """Build the per-chunk schedule: DVE pair-adds and the MM sequence.

    DVE fp8 tensor_add measured ~105 Gelem/s (~156ns per paired output
    tile), so pairing everything makes DVE the bottleneck (95us); the
    PE under its duty-cycle throttle runs ~52-58ns/MM, so pairing
    nothing leaves the PE just below DMA rate (v1's 5us tail). Pair an
    exact PAIR_FRAC of tiles, and schedule each chunk's scratch MMs one
    chunk LATE so the PE never waits on the DVE (the adds get a full
    ~6us chunk period of slack).
    """
    tiles = [p for p in range(CPC) for _ in range(T_pos[p])]
    NT = len(tiles)
    sizes = _chunk_sizes(NT)
    n_ch = len(sizes)

    chunks = []
    a = 0
    seen = 0
    paired = [0, 0]
    pend_mms = []  # scratch MMs deferred to the next chunk
    for ci, sz in enumerate(sizes):
        ct = tiles[a : a + sz]
        runs = []
        s = 0
        for i in range(1, sz + 1):
            if i == sz or ct[i] != ct[s]:
                runs.append((s, i - s, ct[s]))  # local offset, length, class
                s = i
        dve = []  # (engine, out_off, k, in0_off, in1_off)
        mms = list(pend_mms)  # previous chunk's scratch MMs: data ready
        pend_mms = []
        off = 0
        pair = ci < n_ch - UNPAIRED_TAIL
        for s, m, p in runs:
            seen += m
            kv = kg = 0
            if pair:
                want_v = int(PAIR_FRAC_V * seen) - paired[0]
                kv = max(0, min(m // 2, want_v // 2))
                want_g = int(PAIR_FRAC_G * seen) - paired[1]
                kg = max(0, min(m // 2 - kv, want_g // 2))
            paired[0] += 2 * kv
            paired[1] += 2 * kg
            if kv:
                dve.append(("v", off, kv, s, s + kv))
            if kg:
                dve.append(("g", off + kv, kg, s + 2 * kv, s + 2 * kv + kg))
            k2 = kv + kg
            for i in range(s + 2 * (kv + kg), s + m):
                mms.append(("d", ci, i, p))
            for i in range(k2):
                pend_mms.append(("s", ci, off + i, p))
            off += k2
        chunks.append((sz, dve, mms, off))
        a += sz
    if pend_mms:
        chunks[-1] = (
            chunks[-1][0],
            chunks[-1][1],
            chunks[-1][2] + pend_mms,
            chunks[-1][3],
        )
    return sizes, chunks


def _build_module(T_pos):
    import concourse.mybir as mybir
    import concourse.tile as tile
    from concourse import bacc

    f8 = mybir.dt.float8e4
    f32 = mybir.dt.float32

    sizes, chunks = _plan(T_pos)
    max_scr = max(c[3] for c in chunks)

    nc = bacc.Bacc(
        "TRN2",
        target_bir_lowering=False,
        debug=False,
        enable_asserts=False,
        num_devices=CORES,
    )
    emb_ds = [
        nc.dram_tensor(f"emb{i}", [128, sz * D], f8, kind="ExternalInput")
        for i, sz in enumerate(sizes)
    ]
    w_d = nc.dram_tensor("w", [128, 63], f8, kind="ExternalInput")
    out_d = nc.dram_tensor("out", [128, 512], f32, kind="ExternalOutput")

    # global MM order -> (key=(r,l), first/last) for PSUM start/stop flags
    seq = []
    for sz, dve, mms, n_scr in chunks:
        seq.extend(mms)
    first = {}
    last = {}
    layer_last = {}
    for t, (kind, src, off, p) in enumerate(seq):
        key = (t % 4, p // 32)
        first.setdefault(key, t)
        last[key] = t
        layer_last[p // 32] = t
    flush_after = {layer_last[l]: l for l in layer_last}

    from contextlib import ExitStack

    with tile.TileContext(nc) as tc, ExitStack() as stack:
        consts = stack.enter_context(tc.tile_pool(name="consts", bufs=1))
        # one exact-size pool per chunk: a single pool would allocate
        # len(sizes) slots of the max chunk size and overflow SBUF
        epools = [
            stack.enter_context(tc.tile_pool(name=f"ebuf{i}", bufs=1))
            for i in range(len(sizes))
        ]
        scr = stack.enter_context(tc.tile_pool(name="scr", bufs=3))
        psum = stack.enter_context(
            tc.tile_pool(name="psum", bufs=1, space="PSUM")
        )
        outb = stack.enter_context(tc.tile_pool(name="outb", bufs=1))
        if True:
            w_t = consts.tile([128, 63], f8)
            # stationary strip rides the sync ring FIRST (8KB, ~25ns)
            nc.sync.dma_start(out=w_t[:], in_=w_d[:])

            ps_l = [
                psum.tile([128, 512], f32, name=f"ps{i}") for i in range(4)
            ]
            out_t = outb.tile([128, 512], f32)

            ets = []
            for ch, sz in enumerate(sizes):
                et = epools[ch].tile([128, sz * D], f8, tag=f"et{ch}")
                ets.append(et)
                nc.sync.dma_start(out=et[:], in_=emb_ds[ch][:])

            t = 0
            sts = {}
            for ci, (sz, dve, mms, n_scr) in enumerate(chunks):
                et = ets[ci]
                if dve:
                    st = scr.tile([128, max(max_scr, 1) * D], f8, tag="scr")
                    sts[ci] = st
                for eng, out_off, k, i0, i1 in dve:
                    e = nc.vector if eng == "v" else nc.gpsimd
                    e.tensor_add(
                        out=st[:, out_off * D : (out_off + k) * D],
                        in0=et[:, i0 * D : (i0 + k) * D],
                        in1=et[:, i1 * D : (i1 + k) * D],
                    )
                for kind, src_ci, off, p in mms:
                    r = t % 4
                    l = p // 32
                    j32 = p % 32
                    key = (r, l)
                    src = ets[src_ci] if kind == "d" else sts[src_ci]
                    nc.tensor.matmul(
                        ps_l[l][32 * r : 32 * r + 32, 0:128],
                        lhsT=w_t[:, 31 - j32 : 63 - j32],
                        rhs=src[:, off * D : (off + 1) * D],
                        start=(first[key] == t),
                        stop=(last[key] == t),
                        tile_position=(0, 32 * r),
                    )
                    if t in flush_after:
                        l2 = flush_after[t]
                        nc.vector.tensor_copy(
                            out=out_t[:, 128 * l2 : 128 * (l2 + 1)],
                            in_=ps_l[l2][:, 0:128],
                        )
                        nc.scalar.dma_start(
                            out=out_d[:, 128 * l2 : 128 * (l2 + 1)],
                            in_=out_t[:, 128 * l2 : 128 * (l2 + 1)],
                        )
                    t += 1

    nc.compile()
    return nc


def _schedule(counts):
    T_c = -(-counts // 128)  # ceil
    rank = np.argsort(-T_c, kind="stable")  # class ids, tile count descending
    T_pos = T_c[rank[np.arange(CPC) * 8]]  # max of each octet
    return rank, tuple(int(x) for x in T_pos)


def _chunk_sizes(NT):
    # small head (fast first matmul), ~2MB middle chunks (DMA efficiency),
    # small unpaired tail (minimal PE trailing after the last chunk lands).
    head = [4, 28, 92]
    tail = [64, 16, 8, 4]
    mid_total = NT - sum(head) - sum(tail)
    n_mid = max(1, round(mid_total / 132))
    base = mid_total // n_mid
    mid = [base + (1 if i < mid_total % n_mid else 0) for i in range(n_mid)]
    return head + mid + tail


def kernel(embeddings, labels):
    global LAST_RESULT
    from concourse.bass_utils import run_bass_kernel_spmd

    embeddings = np.asarray(embeddings)
    labels = np.asarray(labels).astype(np.int64)
    N = labels.shape[0]

    counts = np.bincount(labels, minlength=C)
    rank, T_pos = _schedule(counts)
    NT = int(sum(T_pos))
    sizes = _chunk_sizes(NT)

    key = T_pos
    if key not in _cache:
        _cache[key] = _build_module(list(T_pos))
    nc = _cache[key]

    # ---- host layout: sorted, class-padded, per-core ----
    embq = embeddings.astype(ml_dtypes.float8_e4m3)
    embq_ext = np.zeros((N + 1, D), dtype=ml_dtypes.float8_e4m3)
    embq_ext[:N] = embq
    order = np.argsort(labels, kind="stable")
    cls_start = np.zeros(C + 1, dtype=np.int64)
    np.cumsum(counts, out=cls_start[1:])

    slot_base = np.zeros(CPC + 1, dtype=np.int64)
    np.cumsum(np.asarray(T_pos, dtype=np.int64) * 128, out=slot_base[1:])

    w = np.zeros((128, 63), dtype=ml_dtypes.float8_e4m3)
    w[:, 31] = 1.0

    in_maps = []
    for k in range(CORES):
        idx = np.full(NT * 128, N, dtype=np.int64)
        for p in range(CPC):
            c = rank[8 * p + k]
            n = counts[c]
            idx[slot_base[p] : slot_base[p] + n] = order[
                cls_start[c] : cls_start[c] + n
            ]
        ec = embq_ext[idx].reshape(NT, 128, D)  # [tile, row, d] fp8
        m = {"w": w}
        a = 0
        for i, sz in enumerate(sizes):
            m[f"emb{i}"] = np.ascontiguousarray(
                ec[a : a + sz].transpose(1, 0, 2)
            ).reshape(128, sz * D)
            a += sz
        in_maps.append(m)

    res = run_bass_kernel_spmd(
        nc,
        in_maps,
        core_ids=list(range(CORES)),
        trace=TRACE,
        **TRACE_KWARGS,
    )
    LAST_RESULT = res

    # ---- host combine: sum 4 colgroup replicas, then means/variance ----
    sums = np.zeros((C, D), dtype=np.float64)
    for k in range(CORES):
        o = res.results[k]["out"].astype(np.float64)
        # [r=4, j32=32, l=4, d=128] -> sum over r -> [l, j32, d] -> [p, d]
        s_all = o.reshape(4, 32, 4, 128).sum(axis=0).transpose(1, 0, 2)
        s_all = s_all.reshape(CPC + 3, D)[:CPC]
        sums[rank[np.arange(CPC) * 8 + k]] = s_all
    means = sums / counts[:, None]
    mu = means.mean(axis=0)
    var = ((means - mu) ** 2).sum(axis=0) / (C - 1)
    return np.float32(-var.mean())
